# revision 13
# baseline (speedup 1.0000x reference)
"""Causal self-attention with RoPE on 8 trn2 NeuronCores.

Sharding: tensor-parallel over heads (Megatron style). 16 heads, 8 cores
-> 2 heads per core. Each core computes q/k/v for its 2 heads, causal
attention, and a partial output projection against its w_o column slice.
Host sums the 8 partial outputs (the Megatron all-reduce, done at gather).

v2: software-pipelined emission. The TRN2 PE clock p-states (2.4 GHz only
after 3us of continuous execution, 1.2 GHz after any idle gap) make PE
gaps extremely expensive, so the kernel is emitted as one interleaved
stream: attention of 512-token block j (ACT-exp / DVE-heavy) is
interleaved at instruction granularity with the QKV projection of block
j+1 and the output projection of block j-1 (both pure PE) as "filler".

Per 512-token block j (8 blocks = 2 batches x 4):
 - Q-pass / K-pass / V-pass: three passes over the SBUF-resident x tiles
   of the block, each accumulating in a small PSUM footprint (2 banks qk,
   1 bank v) so attention + WO psum fits alongside: qk 2 + v/wo 2 +
   scores 3 + av 1 = 8 banks.
 - RoPE applied from a bf16 staging copy of the q/k PSUM, writing
   qhat/khat (per-head [d, t] layout) directly with partition-sliced DVE
   ops (no repack DMAs).
 - Attention per head: per 128-key-tile i: scoresT[ts,tq] single matmul
   (khat_i stationary, qhat_j moving), exp on ACT (scale folded),
   causal 0/1 mask multiply on diagonal tiles; then per 128-query tile:
   AV chain over v tiles with a ones column producing y and the softmax
   denominator in one accumulation; normalize, PE-transpose to yT.
 - WO: per 128-token tile, 4x 512-wide chains over both heads, drained
   alternately on ACT/DVE, DMA'd out as bf16 partials (summed on host).
"""

import math

import numpy as np

B, T, C, H = 2, 2048, 2048, 16
D = C // H  # 128
NCORES = 8
HPC = H // NCORES  # heads per core = 2
N = B * T  # 4096 token rows
NB = T // 512  # 4 blocks of 512 per batch
NBLK = B * NB  # 8 global 512-token blocks
CT = C // 128  # 16 contraction tiles
VW = HPC * D + 2 * HPC  # 260: per t-tile v storage [v_h0|1|pad|v_h1|1|pad]

_COMPILED = None


def _build():
    import concourse.bacc as bacc
    import concourse.mybir as mybir
    import concourse.tile as tile
    from concourse.masks import make_identity

    f32 = mybir.dt.float32
    bf16 = mybir.dt.bfloat16

    nc = bacc.Bacc("TRN2", target_bir_lowering=False, debug=False)
    xT = nc.declare_dram_parameter("xT", [C, N], bf16, isOutput=False)
    w_qk = nc.declare_dram_parameter("w_qk", [C, 4 * D], bf16, isOutput=False)
    w_v = nc.declare_dram_parameter("w_v", [C, HPC * D], bf16, isOutput=False)
    w_o = nc.declare_dram_parameter("w_o", [HPC * D, C], bf16, isOutput=False)
    cos2 = nc.declare_dram_parameter("cos2", [D, N], bf16, isOutput=False)
    sin2 = nc.declare_dram_parameter("sin2", [D, N], bf16, isOutput=False)
    masks = nc.declare_dram_parameter("masks", [128, 4 * 512], bf16, isOutput=False)
    out_p = nc.declare_dram_parameter("out_p", [N, C], bf16, isOutput=True)

    SCALE = 1.0 / math.sqrt(D)

    with tile.TileContext(nc) as tc:
        with (
            tc.tile_pool(name="wpool", bufs=1) as wpool,
            tc.tile_pool(name="xpool", bufs=1) as xpool,
            tc.tile_pool(name="qkh", bufs=1) as qkhpool,
            tc.tile_pool(name="pcp", bufs=2) as pcpool,
            tc.tile_pool(name="rtmp", bufs=4) as rtpool,
            tc.tile_pool(name="expp", bufs=20) as expool,
            tc.tile_pool(name="ysb", bufs=3) as ypool,
            tc.tile_pool(name="rsb", bufs=3) as rpool,
            tc.tile_pool(name="yop", bufs=2) as yopool,
            tc.tile_pool(name="pf", bufs=2, space="PSUM") as pf,
            tc.tile_pool(name="psc", bufs=4, space="PSUM") as psc,
            tc.tile_pool(name="py", bufs=2, space="PSUM") as py,
        ):
            # ---- resident weights / constants ----
            # wqk loaded per part (QE then QO then KE then KO) so each pass
            # only waits on its own quarter arriving.
            wqk_sb = wpool.tile([128, CT * 512], bf16, tag="wqk")
            for pt in range(4):
                nc.sync.dma_start(
                    out=wqk_sb[:, :]
                    .rearrange("p (kt q e) -> p kt q e", kt=CT, q=4)[:, :, pt, :],
                    in_=w_qk.rearrange("(kt p) (q e) -> p kt q e", p=128, q=4)[
                        :, :, pt, :
                    ],
                )
            wv_sb = wpool.tile([128, CT * 256], bf16, tag="wv")
            nc.sync.dma_start(
                out=wv_sb[:, :].rearrange("p (kt e) -> p kt e", kt=CT),
                in_=w_v.rearrange("(kt p) e -> p kt e", p=128),
            )
            cos_sb = wpool.tile([128, N], bf16, tag="cos")
            nc.sync.dma_start(out=cos_sb[:, :], in_=cos2[:, :])
            sin_sb = wpool.tile([128, N], bf16, tag="sin")
            nc.sync.dma_start(out=sin_sb[:, :], in_=sin2[:, :])
            mask_sb = wpool.tile([128, 4 * 512], bf16, tag="mask")
            nc.sync.dma_start(out=mask_sb[:, :], in_=masks[:, :])
            wo_sb = wpool.tile([128, HPC * C], bf16, tag="wo")
            nc.sync.dma_start(
                out=wo_sb[:, :].rearrange("p (kt o) -> p kt o", kt=HPC),
                in_=w_o.rearrange("(kt p) o -> p kt o", p=128),
            )
            ident = wpool.tile([128, 128], bf16, tag="ident")
            make_identity(nc, ident[:, :])
            # preload the ACT exp table off the critical path
            warm = wpool.tile([128, 1], f32, tag="warm")
            nc.vector.memset(warm[:, :], 0.0)
            nc.scalar.activation(
                warm[:, :], warm[:, :], mybir.ActivationFunctionType.Exp, scale=1.0
            )

            # persistent per-batch state
            v_sb = [wpool.tile([128, 4 * NB * VW], bf16, tag=f"vsb{b}", name=f"v_sb{b}") for b in range(B)]
            for b in range(B):
                for tt in range(4 * NB):
                    for h in range(HPC):
                        col = tt * VW + h * 130 + 128
                        nc.vector.memset(v_sb[b][:, col : col + 1], 1.0)
            khat = [
                [wpool.tile([128, T], bf16, tag=f"kh{b}{h}", name=f"khat{b}{h}") for h in range(HPC)]
                for b in range(B)
            ]
            yT = [
                [wpool.tile([128, T], bf16, tag=f"yt{b}{h}", name=f"yT{b}{h}") for h in range(HPC)]
                for b in range(B)
            ]
            # qhat double-buffered by block parity
            qhat = [
                [qkhpool.tile([128, 512], bf16, tag=f"qh{p}{h}", name=f"qhat{p}{h}") for h in range(HPC)]
                for p in range(2)
            ]
            # x tiles double-buffered by block parity: 4 group tiles of
            # [128, 4*512] per block, one DMA each (fewer dispatches).
            xgs = [
                [xpool.tile([128, 4 * 512], bf16, tag=f"x{p}_{g}", name=f"xg{p}_{g}") for g in range(4)]
                for p in range(2)
            ]

            def xsl(p, c):
                return xgs[p][c // 4][:, (c % 4) * 512 : (c % 4 + 1) * 512]

            def prefetch_x(gj):
                p = gj % 2
                for g in range(4):
                    nc.gpsimd.dma_start(
                        out=xgs[p][g][:, :].rearrange("p (kt n) -> p kt n", kt=4),
                        in_=xT[g * 512 : (g + 1) * 512, gj * 512 : (gj + 1) * 512].rearrange(
                            "(kt p) n -> p kt n", p=128
                        ),
                    )

            def rope_apply(pc, gj, dst_of_h):
                """pc = [E(512)|O(512)] bf16 staging; write rotated per-head
                [d,512] into dst_of_h[h] (cols 0:512 of qhat, or the j-block
                cols of khat)."""
                E, O = pc[:, 0:512], pc[:, 512:1024]
                ce = cos_sb[:, gj * 512 : (gj + 1) * 512]
                se = sin_sb[:, gj * 512 : (gj + 1) * 512]
                t1 = rtpool.tile([128, 512], bf16, tag="rt")
                t2 = rtpool.tile([128, 512], bf16, tag="rt")
                nc.vector.tensor_mul(t1[:, :], E, ce)
                nc.vector.tensor_mul(t2[:, :], O, se)
                for h in range(HPC):
                    hb = 64 * h
                    dst, c0 = dst_of_h[h]
                    nc.vector.tensor_sub(
                        dst[0:64, c0 : c0 + 512], t1[hb : hb + 64, :], t2[hb : hb + 64, :]
                    )
                t3 = rtpool.tile([128, 512], bf16, tag="rt")
                t4 = rtpool.tile([128, 512], bf16, tag="rt")
                nc.vector.tensor_mul(t3[:, :], E, se)
                nc.vector.tensor_mul(t4[:, :], O, ce)
                for h in range(HPC):
                    hb = 64 * h
                    dst, c0 = dst_of_h[h]
                    nc.vector.tensor_add(
                        dst[64:128, c0 : c0 + 512], t3[hb : hb + 64, :], t4[hb : hb + 64, :]
                    )

            def qkv_units(gj):
                """PE filler units for block gj's projections. Q/K run as four
                single-PSUM-bank passes (QE, QO, KE, KO); each drains to half
                of a bf16 staging tile, rope fires after the O half."""
                b, j = divmod(gj, NB)
                p = gj % 2
                units = []

                def qk_passes(part0, dst_of_h):
                    # part0 = 0 for Q (QE,QO), 2 for K (KE,KO)
                    pc = pcpool.tile([128, 1024], bf16, tag="pc", name="pc")
                    ps_e = [
                        pf.tile([128, 512], f32, tag="f", name=f"ps_qk{e}")
                        for e in range(2)
                    ]

                    def unit(e, c0):
                        pt = part0 + e
                        for c in range(c0, c0 + 4):
                            wsl = wqk_sb[:, c * 512 + pt * 128 : c * 512 + (pt + 1) * 128]
                            nc.tensor.matmul(
                                ps_e[e][:, :],
                                wsl,
                                xsl(p, c),
                                start=(c == 0),
                                stop=(c == CT - 1),
                            )
                        if c0 + 4 == CT:
                            nc.scalar.copy(pc[:, e * 512 : (e + 1) * 512], ps_e[e][:, :])
                            if e == 1:
                                rope_apply(pc, gj, dst_of_h)
                    return [
                        lambda e=e, c0=c0: unit(e, c0)
                        for e in range(2)
                        for c0 in range(0, CT, 4)
                    ]

                units += qk_passes(0, [(qhat[p][h], 0) for h in range(HPC)])
                units += qk_passes(2, [(khat[b][h], j * 512) for h in range(HPC)])

                def v_unit(tl):
                    ps = pf.tile([128, 256], f32, tag="f", name="ps_v")
                    for c in range(CT):
                        nc.tensor.matmul(
                            ps[:, :],
                            xsl(p, c)[:, tl * 128 : (tl + 1) * 128],
                            wv_sb[:, c * 256 : (c + 1) * 256],
                            start=(c == 0),
                            stop=(c == CT - 1),
                        )
                    base = (j * 4 + tl) * VW
                    for h in range(HPC):
                        nc.scalar.copy(
                            v_sb[b][:, base + h * 130 : base + h * 130 + 128],
                            ps[:, h * 128 : (h + 1) * 128],
                        )
                units += [lambda tl=tl: v_unit(tl) for tl in range(4)]
                return units

            def wo_units(gj):
                """PE filler units for block gj's output projection."""
                b, j = divmod(gj, NB)
                units = []

                def tt_unit(tt, ob2):
                    # two 512-wide o-chains for token tile tt
                    if ob2 == 0:
                        yo = yopool.tile([128, 2048], bf16, tag="yo", name=f"yo_{gj}_{tt}")
                        tt_unit.yo = yo
                    yo = tt_unit.yo
                    tsl = slice((j * 4 + tt) * 128, (j * 4 + tt) * 128 + 128)
                    for ob in (2 * ob2, 2 * ob2 + 1):
                        ps = pf.tile([128, 512], f32, tag="f", name="o_ps")
                        for h in range(HPC):
                            nc.tensor.matmul(
                                ps[:, :],
                                yT[b][h][:, tsl],
                                wo_sb[:, h * C + ob * 512 : h * C + (ob + 1) * 512],
                                start=(h == 0),
                                stop=(h == HPC - 1),
                            )
                        dsl = yo[:, ob * 512 : (ob + 1) * 512]
                        if ob % 2 == 0:
                            nc.scalar.copy(dsl, ps[:, :])
                        else:
                            nc.vector.tensor_copy(dsl, ps[:, :])
                    if ob2 == 1:
                        n0 = b * T + (j * 4 + tt) * 128
                        nc.sync.dma_start(out=out_p[n0 : n0 + 128, :], in_=yo[:, :])
                for tt in range(4):
                    for ob2 in range(2):
                        units.append(lambda tt=tt, ob2=ob2: tt_unit(tt, ob2))
                return units

            def attention(gj, filler):
                """Emit attention for block gj, pulling filler units between
                dependency-stalled PE instructions."""
                b, j = divmod(gj, NB)
                p = gj % 2

                def pull(k=1):
                    for _ in range(k):
                        if filler:
                            filler.popleft()()

                for h in range(HPC):
                    exs = []
                    for i in range(4 * j + 4):
                        sc = psc.tile([128, 512], f32, tag="sc", name="sc")
                        nc.tensor.matmul(
                            sc[:, :],
                            khat[b][h][:, i * 128 : (i + 1) * 128],
                            qhat[p][h][:, :],
                            start=True,
                            stop=True,
                        )
                        ex = expool.tile([128, 512], bf16, tag="ex")
                        nc.scalar.activation(
                            ex[:, :], sc[:, :],
                            mybir.ActivationFunctionType.Exp, scale=SCALE,
                        )
                        pdiag = i - 4 * j
                        if pdiag >= 0:
                            nc.vector.tensor_mul(
                                ex[:, :], ex[:, :],
                                mask_sb[:, pdiag * 512 : (pdiag + 1) * 512],
                            )
                        exs.append(ex)
                        pull()
                    # AV chains; the transpose of tau is deferred until after
                    # tau+1's chain so the PE never head-of-line waits on the
                    # DVE normalize.
                    ysbs = []

                    def transp(tau):
                        yt_ps = psc.tile([128, 128], bf16, tag="sc", name="yt_ps")
                        nc.tensor.transpose(yt_ps[:, :], ysbs[tau][:, :], ident[:, :])
                        gcol = (j * 4 + tau) * 128
                        nc.vector.tensor_copy(yT[b][h][:, gcol : gcol + 128], yt_ps[:, :])

                    for tau in range(4):
                        g = 4 * j + tau
                        y_ps = py.tile([128, 129], f32, tag="y", name="y_ps")
                        for i in range(g + 1):
                            nc.tensor.matmul(
                                y_ps[:, :],
                                exs[i][:, tau * 128 : (tau + 1) * 128],
                                v_sb[b][:, i * VW + h * 130 : i * VW + h * 130 + 129],
                                start=(i == 0),
                                stop=(i == g),
                            )
                            if i % 6 == 5:
                                pull()
                        r = rpool.tile([128, 1], f32, tag="r")
                        nc.vector.reciprocal(r[:, :], y_ps[:, 128:129])
                        y_sb = ypool.tile([128, 128], bf16, tag="y")
                        nc.vector.tensor_scalar_mul(y_sb[:, :], y_ps[:, 0:128], r[:, 0:1])
                        ysbs.append(y_sb)
                        if tau >= 1:
                            transp(tau - 1)
                        pull()
                    transp(3)

            # ---------------- schedule ----------------
            # qdeq: next block's projections — must finish within the window
            # (flushed at window end). wdeq: WO backlog — drained lazily as
            # filler so the tail attention still has PE work, flushed at end.
            from collections import deque

            prefetch_x(0)
            prefetch_x(1)
            for u in qkv_units(0):
                u()
            qdeq = deque()
            wdeq = deque()

            class F:
                # qdeq drains freely; wdeq keeps `reserve` units held back as
                # tail filler for the last (longest) attention window.
                def __init__(self, q, w):
                    self.q, self.w = q, w
                    self.reserve = 12

                def __bool__(self):
                    return bool(self.q) or len(self.w) > self.reserve

                def popleft(self):
                    return self.q.popleft() if self.q else self.w.popleft()

            filler = F(qdeq, wdeq)
            for gj in range(NBLK):
                if gj + 2 < NBLK:
                    prefetch_x(gj + 2)
                if gj + 1 < NBLK:
                    qdeq.extend(qkv_units(gj + 1))
                if gj == NBLK - 1:
                    filler.reserve = 0
                attention(gj, filler)
                while qdeq:
                    qdeq.popleft()()
                wdeq.extend(wo_units(gj))
            while wdeq:
                wdeq.popleft()()
    nc.finalize()
    return nc


def _prep_inputs(x, w_qkv, w_o, rope_cos, rope_sin):
    import ml_dtypes

    bf = ml_dtypes.bfloat16
    xTh = np.ascontiguousarray(x.reshape(N, C).T).astype(bf)
    cosT = np.ascontiguousarray(rope_cos.T)  # [64, T]
    sinT = np.ascontiguousarray(rope_sin.T)
    cos2 = np.tile(np.concatenate([cosT, cosT], 0), (1, B)).astype(bf)
    sin2 = np.tile(np.concatenate([sinT, sinT], 0), (1, B)).astype(bf)

    r = np.arange(128)[:, None]
    c = np.arange(512)[None, :]
    singles = [((c - r) >= 128 * p).astype(np.float32) for p in range(4)]
    mk = np.concatenate(singles, axis=1).astype(bf)

    ev = np.arange(0, D, 2)
    od = np.arange(1, D, 2)
    in_maps = []
    for m in range(NCORES):
        h0, h1 = 2 * m, 2 * m + 1
        # blocks QE|QO|KE|KO; within each, cols = [head0 dims | head1 dims]
        QE = np.concatenate([w_qkv[h0 * D + ev, :], w_qkv[h1 * D + ev, :]], 0).T
        QO = np.concatenate([w_qkv[h0 * D + od, :], w_qkv[h1 * D + od, :]], 0).T
        KE = np.concatenate([w_qkv[C + h0 * D + ev, :], w_qkv[C + h1 * D + ev, :]], 0).T
        KO = np.concatenate([w_qkv[C + h0 * D + od, :], w_qkv[C + h1 * D + od, :]], 0).T
        wqk_m = np.ascontiguousarray(np.concatenate([QE, QO, KE, KO], 1)).astype(bf)
        wv_m = np.ascontiguousarray(
            w_qkv[2 * C + 2 * m * D : 2 * C + (2 * m + 2) * D, :].T
        ).astype(bf)
        wo_m = np.ascontiguousarray(w_o[:, 2 * m * D : (2 * m + 2) * D].T).astype(bf)
        in_maps.append(
            {
                "xT": xTh,
                "w_qk": wqk_m,
                "w_v": wv_m,
                "w_o": wo_m,
                "cos2": cos2,
                "sin2": sin2,
                "masks": np.ascontiguousarray(mk),
            }
        )
    return in_maps


def kernel(x, w_qkv, w_o, rope_cos, rope_sin, _trace=False):
    global _COMPILED
    x = np.asarray(x, dtype=np.float32)
    w_qkv = np.asarray(w_qkv, dtype=np.float32)
    w_o = np.asarray(w_o, dtype=np.float32)
    rope_cos = np.asarray(rope_cos, dtype=np.float32)
    rope_sin = np.asarray(rope_sin, dtype=np.float32)

    from concourse.bass_utils import run_bass_kernel_spmd

    if _COMPILED is None:
        _COMPILED = _build()
    nc = _COMPILED
    in_maps = _prep_inputs(x, w_qkv, w_o, rope_cos, rope_sin)
    res = run_bass_kernel_spmd(
        nc, in_maps, core_ids=list(range(NCORES)), trace=_trace
    )
    out = np.zeros((N, C), dtype=np.float32)
    for m in range(NCORES):
        out += np.asarray(res.results[m]["out_p"], dtype=np.float32)
    kernel._last_results = res
    return out.reshape(B, T, C)


# revision 16
# speedup vs baseline: 1.0040x; 1.0040x over previous
"""Causal self-attention with RoPE on 8 trn2 NeuronCores.

Sharding: tensor-parallel over heads (Megatron style). 16 heads, 8 cores
-> 2 heads per core. Each core computes q/k/v for its 2 heads, causal
attention, and a partial output projection against its w_o column slice.
Host sums the 8 partial outputs (the Megatron all-reduce, done at gather).

v2: software-pipelined emission. The TRN2 PE clock p-states (2.4 GHz only
after 3us of continuous execution, 1.2 GHz after any idle gap) make PE
gaps extremely expensive, so the kernel is emitted as one interleaved
stream: attention of 512-token block j (ACT-exp / DVE-heavy) is
interleaved at instruction granularity with the QKV projection of block
j+1 and the output projection of block j-1 (both pure PE) as "filler".

Per 512-token block j (8 blocks = 2 batches x 4):
 - Q-pass / K-pass / V-pass: three passes over the SBUF-resident x tiles
   of the block, each accumulating in a small PSUM footprint (2 banks qk,
   1 bank v) so attention + WO psum fits alongside: qk 2 + v/wo 2 +
   scores 3 + av 1 = 8 banks.
 - RoPE applied from a bf16 staging copy of the q/k PSUM, writing
   qhat/khat (per-head [d, t] layout) directly with partition-sliced DVE
   ops (no repack DMAs).
 - Attention per head: per 128-key-tile i: scoresT[ts,tq] single matmul
   (khat_i stationary, qhat_j moving), exp on ACT (scale folded),
   causal 0/1 mask multiply on diagonal tiles; then per 128-query tile:
   AV chain over v tiles with a ones column producing y and the softmax
   denominator in one accumulation; normalize, PE-transpose to yT.
 - WO: per 128-token tile, 4x 512-wide chains over both heads, drained
   alternately on ACT/DVE, DMA'd out as bf16 partials (summed on host).
"""

import math

import numpy as np

B, T, C, H = 2, 2048, 2048, 16
D = C // H  # 128
NCORES = 8
HPC = H // NCORES  # heads per core = 2
N = B * T  # 4096 token rows
NB = T // 512  # 4 blocks of 512 per batch
NBLK = B * NB  # 8 global 512-token blocks
CT = C // 128  # 16 contraction tiles
VW = HPC * D + 2 * HPC  # 260: per t-tile v storage [v_h0|1|pad|v_h1|1|pad]

_COMPILED = None


def _build():
    import concourse.bacc as bacc
    import concourse.mybir as mybir
    import concourse.tile as tile
    from concourse.masks import make_identity

    f32 = mybir.dt.float32
    bf16 = mybir.dt.bfloat16

    nc = bacc.Bacc("TRN2", target_bir_lowering=False, debug=False)
    xT = nc.declare_dram_parameter("xT", [C, N], bf16, isOutput=False)
    w_qk = nc.declare_dram_parameter("w_qk", [C, 4 * D], bf16, isOutput=False)
    w_v = nc.declare_dram_parameter("w_v", [C, HPC * D], bf16, isOutput=False)
    w_o = nc.declare_dram_parameter("w_o", [HPC * D, C], bf16, isOutput=False)
    cos2 = nc.declare_dram_parameter("cos2", [D, N], bf16, isOutput=False)
    sin2 = nc.declare_dram_parameter("sin2", [D, N], bf16, isOutput=False)
    masks = nc.declare_dram_parameter("masks", [128, 4 * 512], bf16, isOutput=False)
    out_p = nc.declare_dram_parameter("out_p", [N, C], bf16, isOutput=True)

    SCALE = 1.0 / math.sqrt(D)

    with tile.TileContext(nc) as tc:
        with (
            tc.tile_pool(name="wpool", bufs=1) as wpool,
            tc.tile_pool(name="xpool", bufs=1) as xpool,
            tc.tile_pool(name="qkh", bufs=1) as qkhpool,
            tc.tile_pool(name="pcp", bufs=2) as pcpool,
            tc.tile_pool(name="rtmp", bufs=4) as rtpool,
            tc.tile_pool(name="expp", bufs=20) as expool,
            tc.tile_pool(name="ysb", bufs=3) as ypool,
            tc.tile_pool(name="rsb", bufs=3) as rpool,
            tc.tile_pool(name="yop", bufs=2) as yopool,
            tc.tile_pool(name="pf", bufs=2, space="PSUM") as pf,
            tc.tile_pool(name="psc", bufs=4, space="PSUM") as psc,
            tc.tile_pool(name="py", bufs=2, space="PSUM") as py,
        ):
            # ---- resident weights / constants ----
            # wqk chunked by contraction range so the first pass only waits
            # on the first quarter.
            wqk_sb = wpool.tile([128, CT * 512], bf16, tag="wqk")
            for cc in range(0, CT, 4):
                nc.sync.dma_start(
                    out=wqk_sb[:, cc * 512 : (cc + 4) * 512].rearrange(
                        "p (kt e) -> p kt e", kt=4
                    ),
                    in_=w_qk.rearrange("(kt p) e -> p kt e", p=128)[
                        :, cc : cc + 4, :
                    ],
                )
            wv_sb = wpool.tile([128, CT * 256], bf16, tag="wv")
            nc.sync.dma_start(
                out=wv_sb[:, :].rearrange("p (kt e) -> p kt e", kt=CT),
                in_=w_v.rearrange("(kt p) e -> p kt e", p=128),
            )
            cos_sb = wpool.tile([128, N], bf16, tag="cos")
            nc.sync.dma_start(out=cos_sb[:, :], in_=cos2[:, :])
            sin_sb = wpool.tile([128, N], bf16, tag="sin")
            nc.sync.dma_start(out=sin_sb[:, :], in_=sin2[:, :])
            mask_sb = wpool.tile([128, 4 * 512], bf16, tag="mask")
            nc.sync.dma_start(out=mask_sb[:, :], in_=masks[:, :])
            wo_sb = wpool.tile([128, HPC * C], bf16, tag="wo")
            nc.sync.dma_start(
                out=wo_sb[:, :].rearrange("p (kt o) -> p kt o", kt=HPC),
                in_=w_o.rearrange("(kt p) o -> p kt o", p=128),
            )
            ident = wpool.tile([128, 128], bf16, tag="ident")
            make_identity(nc, ident[:, :])
            # preload the ACT exp table off the critical path
            warm = wpool.tile([128, 1], f32, tag="warm")
            nc.vector.memset(warm[:, :], 0.0)
            nc.scalar.activation(
                warm[:, :], warm[:, :], mybir.ActivationFunctionType.Exp, scale=1.0
            )

            # persistent per-batch state
            v_sb = [wpool.tile([128, 4 * NB * VW], bf16, tag=f"vsb{b}", name=f"v_sb{b}") for b in range(B)]
            for b in range(B):
                for tt in range(4 * NB):
                    for h in range(HPC):
                        col = tt * VW + h * 130 + 128
                        nc.vector.memset(v_sb[b][:, col : col + 1], 1.0)
            khat = [
                [wpool.tile([128, T], bf16, tag=f"kh{b}{h}", name=f"khat{b}{h}") for h in range(HPC)]
                for b in range(B)
            ]
            yT = [
                [wpool.tile([128, T], bf16, tag=f"yt{b}{h}", name=f"yT{b}{h}") for h in range(HPC)]
                for b in range(B)
            ]
            # qhat double-buffered by block parity
            qhat = [
                [qkhpool.tile([128, 512], bf16, tag=f"qh{p}{h}", name=f"qhat{p}{h}") for h in range(HPC)]
                for p in range(2)
            ]
            # x tiles double-buffered by block parity: 4 group tiles of
            # [128, 4*512] per block, one DMA each (fewer dispatches).
            xgs = [
                [xpool.tile([128, 4 * 512], bf16, tag=f"x{p}_{g}", name=f"xg{p}_{g}") for g in range(4)]
                for p in range(2)
            ]

            def xsl(p, c):
                return xgs[p][c // 4][:, (c % 4) * 512 : (c % 4 + 1) * 512]

            def prefetch_x(gj):
                p = gj % 2
                for g in range(4):
                    nc.gpsimd.dma_start(
                        out=xgs[p][g][:, :].rearrange("p (kt n) -> p kt n", kt=4),
                        in_=xT[g * 512 : (g + 1) * 512, gj * 512 : (gj + 1) * 512].rearrange(
                            "(kt p) n -> p kt n", p=128
                        ),
                    )

            def rope_apply(pc, gj, dst_of_h):
                """pc = [E(512)|O(512)] bf16 staging; write rotated per-head
                [d,512] into dst_of_h[h] (cols 0:512 of qhat, or the j-block
                cols of khat)."""
                E, O = pc[:, 0:512], pc[:, 512:1024]
                ce = cos_sb[:, gj * 512 : (gj + 1) * 512]
                se = sin_sb[:, gj * 512 : (gj + 1) * 512]
                t1 = rtpool.tile([128, 512], bf16, tag="rt")
                t2 = rtpool.tile([128, 512], bf16, tag="rt")
                nc.vector.tensor_mul(t1[:, :], E, ce)
                nc.vector.tensor_mul(t2[:, :], O, se)
                for h in range(HPC):
                    hb = 64 * h
                    dst, c0 = dst_of_h[h]
                    nc.vector.tensor_sub(
                        dst[0:64, c0 : c0 + 512], t1[hb : hb + 64, :], t2[hb : hb + 64, :]
                    )
                t3 = rtpool.tile([128, 512], bf16, tag="rt")
                t4 = rtpool.tile([128, 512], bf16, tag="rt")
                nc.vector.tensor_mul(t3[:, :], E, se)
                nc.vector.tensor_mul(t4[:, :], O, ce)
                for h in range(HPC):
                    hb = 64 * h
                    dst, c0 = dst_of_h[h]
                    nc.vector.tensor_add(
                        dst[64:128, c0 : c0 + 512], t3[hb : hb + 64, :], t4[hb : hb + 64, :]
                    )

            def qkv_units(gj):
                """PE filler units for block gj's projections. Q/K run as four
                single-PSUM-bank passes (QE, QO, KE, KO); each drains to half
                of a bf16 staging tile, rope fires after the O half."""
                b, j = divmod(gj, NB)
                p = gj % 2
                units = []

                def qk_passes(part0, dst_of_h):
                    # part0 = 0 for Q (QE,QO), 2 for K (KE,KO)
                    pc = pcpool.tile([128, 1024], bf16, tag="pc", name="pc")
                    ps_e = [
                        pf.tile([128, 512], f32, tag="f", name=f"ps_qk{e}")
                        for e in range(2)
                    ]

                    def unit(e, c0):
                        pt = part0 + e
                        for c in range(c0, c0 + 4):
                            wsl = wqk_sb[:, c * 512 + pt * 128 : c * 512 + (pt + 1) * 128]
                            nc.tensor.matmul(
                                ps_e[e][:, :],
                                wsl,
                                xsl(p, c),
                                start=(c == 0),
                                stop=(c == CT - 1),
                            )
                        if c0 + 4 == CT:
                            nc.scalar.copy(pc[:, e * 512 : (e + 1) * 512], ps_e[e][:, :])
                            if e == 1:
                                rope_apply(pc, gj, dst_of_h)
                    return [
                        lambda e=e, c0=c0: unit(e, c0)
                        for e in range(2)
                        for c0 in range(0, CT, 4)
                    ]

                # K first: the K -> rope -> khat chain is the next window's
                # scores critical path, so retire it early in this window.
                units += qk_passes(2, [(khat[b][h], j * 512) for h in range(HPC)])
                units += qk_passes(0, [(qhat[p][h], 0) for h in range(HPC)])

                def v_unit(tl):
                    ps = pf.tile([128, 256], f32, tag="f", name="ps_v")
                    for c in range(CT):
                        nc.tensor.matmul(
                            ps[:, :],
                            xsl(p, c)[:, tl * 128 : (tl + 1) * 128],
                            wv_sb[:, c * 256 : (c + 1) * 256],
                            start=(c == 0),
                            stop=(c == CT - 1),
                        )
                    base = (j * 4 + tl) * VW
                    for h in range(HPC):
                        nc.scalar.copy(
                            v_sb[b][:, base + h * 130 : base + h * 130 + 128],
                            ps[:, h * 128 : (h + 1) * 128],
                        )
                units += [lambda tl=tl: v_unit(tl) for tl in range(4)]
                return units

            def wo_units(gj):
                """PE filler units for block gj's output projection."""
                b, j = divmod(gj, NB)
                units = []

                def tt_unit(tt, ob2):
                    # two 512-wide o-chains for token tile tt
                    if ob2 == 0:
                        yo = yopool.tile([128, 2048], bf16, tag="yo", name=f"yo_{gj}_{tt}")
                        tt_unit.yo = yo
                    yo = tt_unit.yo
                    tsl = slice((j * 4 + tt) * 128, (j * 4 + tt) * 128 + 128)
                    for ob in (2 * ob2, 2 * ob2 + 1):
                        ps = pf.tile([128, 512], f32, tag="f", name="o_ps")
                        for h in range(HPC):
                            nc.tensor.matmul(
                                ps[:, :],
                                yT[b][h][:, tsl],
                                wo_sb[:, h * C + ob * 512 : h * C + (ob + 1) * 512],
                                start=(h == 0),
                                stop=(h == HPC - 1),
                            )
                        dsl = yo[:, ob * 512 : (ob + 1) * 512]
                        if ob % 2 == 0:
                            nc.scalar.copy(dsl, ps[:, :])
                        else:
                            nc.vector.tensor_copy(dsl, ps[:, :])
                    if ob2 == 1:
                        n0 = b * T + (j * 4 + tt) * 128
                        nc.sync.dma_start(out=out_p[n0 : n0 + 128, :], in_=yo[:, :])
                for tt in range(4):
                    for ob2 in range(2):
                        units.append(lambda tt=tt, ob2=ob2: tt_unit(tt, ob2))
                return units

            def attention(gj, filler):
                """Emit attention for block gj, pulling filler units between
                dependency-stalled PE instructions."""
                b, j = divmod(gj, NB)
                p = gj % 2

                def pull(k=1):
                    for _ in range(k):
                        if filler:
                            filler.popleft()()

                for h in range(HPC):
                    exs = []
                    for i in range(4 * j + 4):
                        sc = psc.tile([128, 512], f32, tag="sc", name="sc")
                        nc.tensor.matmul(
                            sc[:, :],
                            khat[b][h][:, i * 128 : (i + 1) * 128],
                            qhat[p][h][:, :],
                            start=True,
                            stop=True,
                        )
                        ex = expool.tile([128, 512], bf16, tag="ex")
                        nc.scalar.activation(
                            ex[:, :], sc[:, :],
                            mybir.ActivationFunctionType.Exp, scale=SCALE,
                        )
                        pdiag = i - 4 * j
                        if pdiag >= 0:
                            nc.vector.tensor_mul(
                                ex[:, :], ex[:, :],
                                mask_sb[:, pdiag * 512 : (pdiag + 1) * 512],
                            )
                        exs.append(ex)
                        pull()
                    # AV chains; the transpose of tau is deferred until after
                    # tau+1's chain so the PE never head-of-line waits on the
                    # DVE normalize.
                    ysbs = []

                    def transp(tau):
                        yt_ps = psc.tile([128, 128], bf16, tag="sc", name="yt_ps")
                        nc.tensor.transpose(yt_ps[:, :], ysbs[tau][:, :], ident[:, :])
                        gcol = (j * 4 + tau) * 128
                        nc.vector.tensor_copy(yT[b][h][:, gcol : gcol + 128], yt_ps[:, :])

                    for tau in range(4):
                        g = 4 * j + tau
                        y_ps = py.tile([128, 129], f32, tag="y", name="y_ps")
                        for i in range(g + 1):
                            nc.tensor.matmul(
                                y_ps[:, :],
                                exs[i][:, tau * 128 : (tau + 1) * 128],
                                v_sb[b][:, i * VW + h * 130 : i * VW + h * 130 + 129],
                                start=(i == 0),
                                stop=(i == g),
                            )
                            if i % 6 == 5:
                                pull()
                        r = rpool.tile([128, 1], f32, tag="r")
                        nc.vector.reciprocal(r[:, :], y_ps[:, 128:129])
                        y_sb = ypool.tile([128, 128], bf16, tag="y")
                        nc.vector.tensor_scalar_mul(y_sb[:, :], y_ps[:, 0:128], r[:, 0:1])
                        ysbs.append(y_sb)
                        if tau >= 1:
                            transp(tau - 1)
                        pull()
                    transp(3)

            # ---------------- schedule ----------------
            # qdeq: next block's projections — must finish within the window
            # (flushed at window end). wdeq: WO backlog — drained lazily as
            # filler so the tail attention still has PE work, flushed at end.
            from collections import deque

            prefetch_x(0)
            prefetch_x(1)
            for u in qkv_units(0):
                u()
            qdeq = deque()
            wdeq = deque()

            class F:
                # qdeq drains freely; wdeq keeps `reserve` units held back as
                # tail filler for the last (longest) attention window.
                def __init__(self, q, w):
                    self.q, self.w = q, w
                    self.reserve = 16

                def __bool__(self):
                    return bool(self.q) or len(self.w) > self.reserve

                def popleft(self):
                    return self.q.popleft() if self.q else self.w.popleft()

            filler = F(qdeq, wdeq)
            for gj in range(NBLK):
                if gj + 2 < NBLK:
                    prefetch_x(gj + 2)
                if gj + 1 < NBLK:
                    qdeq.extend(qkv_units(gj + 1))
                if gj == NBLK - 1:
                    filler.reserve = 0
                attention(gj, filler)
                while qdeq:
                    qdeq.popleft()()
                wdeq.extend(wo_units(gj))
            while wdeq:
                wdeq.popleft()()
    nc.finalize()
    return nc


def _prep_inputs(x, w_qkv, w_o, rope_cos, rope_sin):
    import ml_dtypes

    bf = ml_dtypes.bfloat16
    xTh = np.ascontiguousarray(x.reshape(N, C).T).astype(bf)
    cosT = np.ascontiguousarray(rope_cos.T)  # [64, T]
    sinT = np.ascontiguousarray(rope_sin.T)
    cos2 = np.tile(np.concatenate([cosT, cosT], 0), (1, B)).astype(bf)
    sin2 = np.tile(np.concatenate([sinT, sinT], 0), (1, B)).astype(bf)

    r = np.arange(128)[:, None]
    c = np.arange(512)[None, :]
    singles = [((c - r) >= 128 * p).astype(np.float32) for p in range(4)]
    mk = np.concatenate(singles, axis=1).astype(bf)

    ev = np.arange(0, D, 2)
    od = np.arange(1, D, 2)
    in_maps = []
    for m in range(NCORES):
        h0, h1 = 2 * m, 2 * m + 1
        # blocks QE|QO|KE|KO; within each, cols = [head0 dims | head1 dims]
        QE = np.concatenate([w_qkv[h0 * D + ev, :], w_qkv[h1 * D + ev, :]], 0).T
        QO = np.concatenate([w_qkv[h0 * D + od, :], w_qkv[h1 * D + od, :]], 0).T
        KE = np.concatenate([w_qkv[C + h0 * D + ev, :], w_qkv[C + h1 * D + ev, :]], 0).T
        KO = np.concatenate([w_qkv[C + h0 * D + od, :], w_qkv[C + h1 * D + od, :]], 0).T
        wqk_m = np.ascontiguousarray(np.concatenate([QE, QO, KE, KO], 1)).astype(bf)
        wv_m = np.ascontiguousarray(
            w_qkv[2 * C + 2 * m * D : 2 * C + (2 * m + 2) * D, :].T
        ).astype(bf)
        wo_m = np.ascontiguousarray(w_o[:, 2 * m * D : (2 * m + 2) * D].T).astype(bf)
        in_maps.append(
            {
                "xT": xTh,
                "w_qk": wqk_m,
                "w_v": wv_m,
                "w_o": wo_m,
                "cos2": cos2,
                "sin2": sin2,
                "masks": np.ascontiguousarray(mk),
            }
        )
    return in_maps


def kernel(x, w_qkv, w_o, rope_cos, rope_sin, _trace=False):
    global _COMPILED
    x = np.asarray(x, dtype=np.float32)
    w_qkv = np.asarray(w_qkv, dtype=np.float32)
    w_o = np.asarray(w_o, dtype=np.float32)
    rope_cos = np.asarray(rope_cos, dtype=np.float32)
    rope_sin = np.asarray(rope_sin, dtype=np.float32)

    from concourse.bass_utils import run_bass_kernel_spmd

    if _COMPILED is None:
        _COMPILED = _build()
    nc = _COMPILED
    in_maps = _prep_inputs(x, w_qkv, w_o, rope_cos, rope_sin)
    res = run_bass_kernel_spmd(
        nc, in_maps, core_ids=list(range(NCORES)), trace=_trace
    )
    out = np.zeros((N, C), dtype=np.float32)
    for m in range(NCORES):
        out += np.asarray(res.results[m]["out_p"], dtype=np.float32)
    kernel._last_results = res
    return out.reshape(B, T, C)
